# revision 1
# baseline (speedup 1.0000x reference)
"""MinGRU Trainium2 kernel.

Problem: x (8, 4096, 1024) fp32; Wz, Wh (1024, 1024); bz, bh (1024,).
    k = x @ Wz.T + bz ; z = sigmoid(k)
    p = x @ Wh.T + bh ; g = where(p >= 0, p + 0.5, sigmoid(p))
    h_t = (1 - z_t) * h_{t-1} + z_t * g_t   (h_0 = 0.5)
The reference computes this recurrence with a log-space parallel scan; here it
is computed directly in linear space (mathematically identical), using the DVE
TensorTensorScanArith instruction along the free axis.

Sharding: data-parallel over batch, one batch element per NeuronCore (8 cores).

Per-core layout: everything lives transposed, H on partitions, S on the free
axis.  k/p tiles (128, 512) come out of PSUM from 8-step K-accumulated fp32
matmuls; bias adds are fused into the ScalarE activations (per-partition bias);
g = min(sigmoid(p+bh), 0.5) + relu(p+bh) (identical to the where() branch).
"""

import os
import sys

import numpy as np

for _p in ("/opt/trn_rl_repo", "/root/.axon_site/_ro/trn_rl_repo"):
    if os.path.isdir(_p) and _p not in sys.path:
        sys.path.insert(0, _p)

import concourse.bass as bass  # noqa: E402
import concourse.mybir as mybir  # noqa: E402
import concourse.tile as tile  # noqa: E402
from concourse import bacc  # noqa: E402
from concourse.bass_utils import run_bass_kernel_spmd  # noqa: E402

F32 = mybir.dt.float32
N_CORES = 8
B, S, D, H = 8, 4096, 1024, 1024
TS = 512  # sequence strip width (= fp32 matmul max moving free dim)
NK = D // 128
NM = H // 128

_cache: dict = {}


def build_nc(seq_len: int = S, n_cores: int = N_CORES):
    """Build and compile the per-core Bass module (SPMD, identical program)."""
    nt = seq_len // TS
    nc = bacc.Bacc(
        "TRN2", target_bir_lowering=False, debug=False, num_devices=n_cores
    )

    xT_d = nc.dram_tensor("xT", [D, seq_len], F32, kind="ExternalInput")
    wzT_d = nc.dram_tensor("wzT", [D, H], F32, kind="ExternalInput")
    whT_d = nc.dram_tensor("whT", [D, H], F32, kind="ExternalInput")
    bz_d = nc.dram_tensor("bz", [H], F32, kind="ExternalInput")
    bh_d = nc.dram_tensor("bh", [H], F32, kind="ExternalInput")
    hT_d = nc.dram_tensor("hT", [H, seq_len], F32, kind="ExternalOutput")

    AF = mybir.ActivationFunctionType
    OP = mybir.AluOpType

    with tile.TileContext(nc) as tc:
        with (
            tc.tile_pool(name="singles", bufs=1) as singles,
            tc.tile_pool(name="xs", bufs=3) as xpool,
            tc.tile_pool(name="work", bufs=3) as work,
            tc.tile_pool(name="hbuf", bufs=2) as hpool,
            tc.tile_pool(name="psum", bufs=2, space="PSUM") as psum,
        ):
            # Weights resident in SBUF for the whole kernel: (d-part, h-free)
            wz_sb, wh_sb = [], []
            for k in range(NK):
                wz = singles.tile([128, H], F32, tag=f"wz{k}")
                nc.sync.dma_start(out=wz, in_=wzT_d.ap()[k * 128:(k + 1) * 128, :])
                wz_sb.append(wz)
                wh = singles.tile([128, H], F32, tag=f"wh{k}")
                nc.sync.dma_start(out=wh, in_=whT_d.ap()[k * 128:(k + 1) * 128, :])
                wh_sb.append(wh)
            # Biases as (128, NM): column m = bias slice for h-tile m
            bz_sb = singles.tile([128, NM], F32, tag="bz")
            nc.sync.dma_start(out=bz_sb, in_=bz_d.ap().rearrange("(m p) -> p m", p=128))
            bh_sb = singles.tile([128, NM], F32, tag="bh")
            nc.sync.dma_start(out=bh_sb, in_=bh_d.ap().rearrange("(m p) -> p m", p=128))

            h_prev: list = [None] * NM
            for s in range(nt):
                ts_sl = slice(s * TS, (s + 1) * TS)
                xs = []
                for k in range(NK):
                    xt = xpool.tile([128, TS], F32, tag=f"xs{k}")
                    nc.sync.dma_start(
                        out=xt, in_=xT_d.ap()[k * 128:(k + 1) * 128, ts_sl]
                    )
                    xs.append(xt)
                for m in range(NM):
                    m_sl = slice(m * 128, (m + 1) * 128)
                    kp = psum.tile([128, TS], F32, tag="kp")
                    pp = psum.tile([128, TS], F32, tag="pp")
                    for k in range(NK):
                        nc.tensor.matmul(
                            kp[:],
                            lhsT=wz_sb[k][:, m_sl],
                            rhs=xs[k][:],
                            start=(k == 0),
                            stop=(k == NK - 1),
                        )
                    for k in range(NK):
                        nc.tensor.matmul(
                            pp[:],
                            lhsT=wh_sb[k][:, m_sl],
                            rhs=xs[k][:],
                            start=(k == 0),
                            stop=(k == NK - 1),
                        )
                    z = work.tile([128, TS], F32, tag="z")
                    nc.scalar.activation(
                        out=z[:], in_=kp[:], func=AF.Sigmoid, bias=bz_sb[:, m:m + 1]
                    )
                    sp = work.tile([128, TS], F32, tag="sp")
                    nc.scalar.activation(
                        out=sp[:], in_=pp[:], func=AF.Sigmoid, bias=bh_sb[:, m:m + 1]
                    )
                    rp = work.tile([128, TS], F32, tag="rp")
                    nc.scalar.activation(
                        out=rp[:], in_=pp[:], func=AF.Relu, bias=bh_sb[:, m:m + 1]
                    )
                    # a = 1 - z
                    a = work.tile([128, TS], F32, tag="a")
                    nc.vector.tensor_scalar(
                        out=a[:], in0=z[:], scalar1=-1.0, scalar2=1.0,
                        op0=OP.mult, op1=OP.add,
                    )
                    # g = min(sigmoid(p+bh), 0.5) + relu(p+bh)
                    g = work.tile([128, TS], F32, tag="g")
                    nc.vector.scalar_tensor_tensor(
                        out=g[:], in0=sp[:], scalar=0.5, in1=rp[:],
                        op0=OP.min, op1=OP.add,
                    )
                    # b = z * g
                    b = work.tile([128, TS], F32, tag="b")
                    nc.vector.tensor_tensor(out=b[:], in0=z[:], in1=g[:], op=OP.mult)
                    # h_t = a_t * h_{t-1} + b_t along the free axis
                    h = hpool.tile([128, TS], F32, tag=f"h{m}")
                    init = 0.5 if s == 0 else h_prev[m][:, TS - 1:TS]
                    nc.vector.tensor_tensor_scan(
                        out=h[:], data0=a[:], data1=b[:], initial=init,
                        op0=OP.mult, op1=OP.add,
                    )
                    h_prev[m] = h
                    nc.sync.dma_start(out=hT_d.ap()[m_sl, ts_sl], in_=h[:])

    nc.compile()
    return nc


def kernel(x, Wz, bz, Wh, bh):
    x = np.ascontiguousarray(x, dtype=np.float32)
    key = "nc"
    if key not in _cache:
        _cache[key] = build_nc()
    nc = _cache[key]

    wzT = np.ascontiguousarray(Wz.T.astype(np.float32))
    whT = np.ascontiguousarray(Wh.T.astype(np.float32))
    bz = np.ascontiguousarray(bz, dtype=np.float32)
    bh = np.ascontiguousarray(bh, dtype=np.float32)
    in_maps = [
        {
            "xT": np.ascontiguousarray(x[b].T),
            "wzT": wzT,
            "whT": whT,
            "bz": bz,
            "bh": bh,
        }
        for b in range(N_CORES)
    ]
    res = run_bass_kernel_spmd(nc, in_maps, list(range(N_CORES)))
    out = np.empty((B, S, H), np.float32)
    for b in range(N_CORES):
        out[b] = res.results[b]["hT"].T
    return out


# revision 2
# speedup vs baseline: 253.2594x; 253.2594x over previous
"""MinGRU Trainium2 kernel.

Problem: x (8, 4096, 1024) fp32; Wz, Wh (1024, 1024); bz, bh (1024,).
    k = x @ Wz.T + bz ; z = sigmoid(k)
    p = x @ Wh.T + bh ; g = where(p >= 0, p + 0.5, sigmoid(p))
    h_t = (1 - z_t) * h_{t-1} + z_t * g_t   (h_0 = 0.5)
The reference computes this recurrence with a log-space parallel scan; here it
is computed directly in linear space (mathematically identical), using the DVE
TensorTensorScanArith instruction along the free axis.

Sharding: data-parallel over batch, one batch element per NeuronCore (8 cores).

Per-core layout: everything lives transposed, H on partitions, S on the free
axis.  k/p tiles (128, 512) come out of PSUM from 8-step K-accumulated fp32
matmuls; bias adds are fused into the ScalarE activations (per-partition bias);
g = min(sigmoid(p+bh), 0.5) + relu(p+bh) (identical to the where() branch).
"""

import os
import sys

import numpy as np

for _p in ("/opt/trn_rl_repo", "/root/.axon_site/_ro/trn_rl_repo"):
    if os.path.isdir(_p) and _p not in sys.path:
        sys.path.insert(0, _p)

import concourse.bass as bass  # noqa: E402
import concourse.mybir as mybir  # noqa: E402
import concourse.tile as tile  # noqa: E402
from concourse import bacc  # noqa: E402
from concourse.bass_utils import run_bass_kernel_spmd  # noqa: E402

F32 = mybir.dt.float32
F32R = mybir.dt.float32r  # fp32 bits, full-rate PE streaming mode
N_CORES = 8
B, S, D, H = 8, 4096, 1024, 1024
TS = 512  # sequence strip width (= fp32 matmul max moving free dim)
NK = D // 128
NM = H // 128

_cache: dict = {}


def build_nc(seq_len: int = S, n_cores: int = N_CORES):
    """Build and compile the per-core Bass module (SPMD, identical program)."""
    nt = seq_len // TS
    nc = bacc.Bacc(
        "TRN2", target_bir_lowering=False, debug=False, num_devices=n_cores
    )

    xT_d = nc.dram_tensor("xT", [D, seq_len], F32R, kind="ExternalInput")
    wzT_d = nc.dram_tensor("wzT", [D, H], F32R, kind="ExternalInput")
    whT_d = nc.dram_tensor("whT", [D, H], F32R, kind="ExternalInput")
    bz_d = nc.dram_tensor("bz", [H], F32, kind="ExternalInput")
    bh_d = nc.dram_tensor("bh", [H], F32, kind="ExternalInput")
    hT_d = nc.dram_tensor("hT", [H, seq_len], F32, kind="ExternalOutput")

    AF = mybir.ActivationFunctionType
    OP = mybir.AluOpType

    with tile.TileContext(nc) as tc:
        with (
            tc.tile_pool(name="singles", bufs=1) as singles,
            tc.tile_pool(name="xs", bufs=3) as xpool,
            tc.tile_pool(name="work", bufs=3) as work,
            tc.tile_pool(name="hbuf", bufs=2) as hpool,
            tc.tile_pool(name="psum", bufs=2, space="PSUM") as psum,
        ):
            # Weights resident in SBUF for the whole kernel: (d-part, h-free)
            wz_sb, wh_sb = [], []
            for k in range(NK):
                wz = singles.tile([128, H], F32R, tag=f"wz{k}")
                nc.sync.dma_start(out=wz, in_=wzT_d.ap()[k * 128:(k + 1) * 128, :])
                wz_sb.append(wz)
                wh = singles.tile([128, H], F32R, tag=f"wh{k}")
                nc.sync.dma_start(out=wh, in_=whT_d.ap()[k * 128:(k + 1) * 128, :])
                wh_sb.append(wh)
            # Biases as (128, NM): column m = bias slice for h-tile m
            bz_sb = singles.tile([128, NM], F32, tag="bz")
            nc.sync.dma_start(out=bz_sb, in_=bz_d.ap().rearrange("(m p) -> p m", p=128))
            bh_sb = singles.tile([128, NM], F32, tag="bh")
            nc.sync.dma_start(out=bh_sb, in_=bh_d.ap().rearrange("(m p) -> p m", p=128))

            h_prev: list = [None] * NM
            for s in range(nt):
                ts_sl = slice(s * TS, (s + 1) * TS)
                xs = []
                for k in range(NK):
                    xt = xpool.tile([128, TS], F32R, tag=f"xs{k}")
                    nc.sync.dma_start(
                        out=xt, in_=xT_d.ap()[k * 128:(k + 1) * 128, ts_sl]
                    )
                    xs.append(xt)
                for m in range(NM):
                    m_sl = slice(m * 128, (m + 1) * 128)
                    kp = psum.tile([128, TS], F32, tag="kp")
                    pp = psum.tile([128, TS], F32, tag="pp")
                    for k in range(NK):
                        nc.tensor.matmul(
                            kp[:],
                            lhsT=wz_sb[k][:, m_sl],
                            rhs=xs[k][:],
                            start=(k == 0),
                            stop=(k == NK - 1),
                        )
                    for k in range(NK):
                        nc.tensor.matmul(
                            pp[:],
                            lhsT=wh_sb[k][:, m_sl],
                            rhs=xs[k][:],
                            start=(k == 0),
                            stop=(k == NK - 1),
                        )
                    z = work.tile([128, TS], F32, tag="z")
                    nc.scalar.activation(
                        out=z[:], in_=kp[:], func=AF.Sigmoid, bias=bz_sb[:, m:m + 1]
                    )
                    sp = work.tile([128, TS], F32, tag="sp")
                    nc.scalar.activation(
                        out=sp[:], in_=pp[:], func=AF.Sigmoid, bias=bh_sb[:, m:m + 1]
                    )
                    rp = work.tile([128, TS], F32, tag="rp")
                    nc.scalar.activation(
                        out=rp[:], in_=pp[:], func=AF.Relu, bias=bh_sb[:, m:m + 1]
                    )
                    # a = 1 - z
                    a = work.tile([128, TS], F32, tag="a")
                    nc.vector.tensor_scalar(
                        out=a[:], in0=z[:], scalar1=-1.0, scalar2=1.0,
                        op0=OP.mult, op1=OP.add,
                    )
                    # g = min(sigmoid(p+bh), 0.5) + relu(p+bh)
                    g = work.tile([128, TS], F32, tag="g")
                    nc.vector.scalar_tensor_tensor(
                        out=g[:], in0=sp[:], scalar=0.5, in1=rp[:],
                        op0=OP.min, op1=OP.add,
                    )
                    # b = z * g
                    b = work.tile([128, TS], F32, tag="b")
                    nc.vector.tensor_tensor(out=b[:], in0=z[:], in1=g[:], op=OP.mult)
                    # h_t = a_t * h_{t-1} + b_t along the free axis
                    h = hpool.tile([128, TS], F32, tag=f"h{m}")
                    init = 0.5 if s == 0 else h_prev[m][:, TS - 1:TS]
                    nc.vector.tensor_tensor_scan(
                        out=h[:], data0=a[:], data1=b[:], initial=init,
                        op0=OP.mult, op1=OP.add,
                    )
                    h_prev[m] = h
                    nc.sync.dma_start(out=hT_d.ap()[m_sl, ts_sl], in_=h[:])

    nc.compile()
    return nc


def kernel(x, Wz, bz, Wh, bh):
    x = np.ascontiguousarray(x, dtype=np.float32)
    key = "nc"
    if key not in _cache:
        _cache[key] = build_nc()
    nc = _cache[key]

    wzT = np.ascontiguousarray(Wz.T.astype(np.float32))
    whT = np.ascontiguousarray(Wh.T.astype(np.float32))
    bz = np.ascontiguousarray(bz, dtype=np.float32)
    bh = np.ascontiguousarray(bh, dtype=np.float32)
    in_maps = [
        {
            "xT": np.ascontiguousarray(x[b].T),
            "wzT": wzT,
            "whT": whT,
            "bz": bz,
            "bh": bh,
        }
        for b in range(N_CORES)
    ]
    res = run_bass_kernel_spmd(nc, in_maps, list(range(N_CORES)))
    out = np.empty((B, S, H), np.float32)
    for b in range(N_CORES):
        out[b] = res.results[b]["hT"].T
    return out


# revision 3
# speedup vs baseline: 258.4645x; 1.0206x over previous
"""MinGRU Trainium2 kernel.

Problem: x (8, 4096, 1024) fp32; Wz, Wh (1024, 1024); bz, bh (1024,).
    k = x @ Wz.T + bz ; z = sigmoid(k)
    p = x @ Wh.T + bh ; g = where(p >= 0, p + 0.5, sigmoid(p))
    h_t = (1 - z_t) * h_{t-1} + z_t * g_t   (h_0 = 0.5)
The reference computes this recurrence with a log-space parallel scan; here it
is computed directly in linear space (mathematically identical), using the DVE
TensorTensorScanArith instruction along the free axis.

Sharding: data-parallel over batch, one batch element per NeuronCore (8 cores).

Per-core layout: everything lives transposed, H on partitions, S on the free
axis.  k/p tiles (128, 512) come out of PSUM from 8-step K-accumulated
float32r matmuls (fp32 bits, full-rate PE streaming); bias adds are fused into
the ScalarE activations (per-partition bias); g = min(sigmoid(p+bh), 0.5) +
relu(p+bh) (identical to the where() branch).  b = z*g runs on the otherwise
idle GpSimd engine to keep the DVE below the PE roofline.
"""

import os
import sys

import numpy as np

for _p in ("/opt/trn_rl_repo", "/root/.axon_site/_ro/trn_rl_repo"):
    if os.path.isdir(_p) and _p not in sys.path:
        sys.path.insert(0, _p)

import concourse.bass as bass  # noqa: E402
import concourse.mybir as mybir  # noqa: E402
import concourse.tile as tile  # noqa: E402
from concourse import bacc  # noqa: E402
from concourse.bass_utils import run_bass_kernel_spmd  # noqa: E402

F32 = mybir.dt.float32
F32R = mybir.dt.float32r  # fp32 bits, full-rate PE streaming mode
N_CORES = 8
B, S, D, H = 8, 4096, 1024, 1024
TS = 512  # sequence strip width (= fp32 matmul max moving free dim)
NK = D // 128
NM = H // 128

_cache: dict = {}


def build_nc(seq_len: int = S, n_cores: int = N_CORES):
    """Build and compile the per-core Bass module (SPMD, identical program)."""
    nt = seq_len // TS
    nc = bacc.Bacc(
        "TRN2", target_bir_lowering=False, debug=False, num_devices=n_cores
    )

    xT_d = nc.dram_tensor("xT", [D, seq_len], F32R, kind="ExternalInput")
    wzT_d = nc.dram_tensor("wzT", [D, H], F32R, kind="ExternalInput")
    whT_d = nc.dram_tensor("whT", [D, H], F32R, kind="ExternalInput")
    bz_d = nc.dram_tensor("bz", [H], F32, kind="ExternalInput")
    bh_d = nc.dram_tensor("bh", [H], F32, kind="ExternalInput")
    hT_d = nc.dram_tensor("hT", [H, seq_len], F32, kind="ExternalOutput")

    AF = mybir.ActivationFunctionType
    OP = mybir.AluOpType

    with tile.TileContext(nc) as tc:
        with (
            tc.tile_pool(name="singles", bufs=1) as singles,
            tc.tile_pool(name="xs", bufs=3) as xpool,
            tc.tile_pool(name="work", bufs=3) as work,
            tc.tile_pool(name="hbuf", bufs=2) as hpool,
            tc.tile_pool(name="psum", bufs=3, space="PSUM") as psum,
        ):
            # First strip of x before the weights: the first matmuls need
            # xs(s=0) + the m<4 half of the weights, so order the initial DMAs
            # to unblock the PE as early as possible.
            xs0 = []
            for k in range(NK):
                xt = xpool.tile([128, TS], F32R, tag=f"xs{k}")
                nc.sync.dma_start(out=xt, in_=xT_d.ap()[k * 128:(k + 1) * 128, 0:TS])
                xs0.append(xt)
            # Weights resident in SBUF, split in half along H so the m-tiles
            # 0..3 unblock after the first 16 chunk loads: wz_sb[k][j] holds
            # WzT[k*128:(k+1)*128, j*512:(j+1)*512].
            wz_sb = [[None, None] for _ in range(NK)]
            wh_sb = [[None, None] for _ in range(NK)]
            for j in range(2):
                for k in range(NK):
                    wz = singles.tile([128, H // 2], F32R, tag=f"wz{k}_{j}")
                    nc.sync.dma_start(
                        out=wz,
                        in_=wzT_d.ap()[k * 128:(k + 1) * 128,
                                       j * (H // 2):(j + 1) * (H // 2)],
                    )
                    wz_sb[k][j] = wz
                for k in range(NK):
                    wh = singles.tile([128, H // 2], F32R, tag=f"wh{k}_{j}")
                    nc.sync.dma_start(
                        out=wh,
                        in_=whT_d.ap()[k * 128:(k + 1) * 128,
                                       j * (H // 2):(j + 1) * (H // 2)],
                    )
                    wh_sb[k][j] = wh
            # Biases as (128, NM): column m = bias slice for h-tile m
            bz_sb = singles.tile([128, NM], F32, tag="bz")
            nc.sync.dma_start(out=bz_sb, in_=bz_d.ap().rearrange("(m p) -> p m", p=128))
            bh_sb = singles.tile([128, NM], F32, tag="bh")
            nc.sync.dma_start(out=bh_sb, in_=bh_d.ap().rearrange("(m p) -> p m", p=128))

            h_prev: list = [None] * NM
            for s in range(nt):
                ts_sl = slice(s * TS, (s + 1) * TS)
                if s == 0:
                    xs = xs0
                else:
                    xs = []
                    for k in range(NK):
                        xt = xpool.tile([128, TS], F32R, tag=f"xs{k}")
                        nc.sync.dma_start(
                            out=xt, in_=xT_d.ap()[k * 128:(k + 1) * 128, ts_sl]
                        )
                        xs.append(xt)
                for m in range(NM):
                    j, mj = divmod(m, NM // 2)
                    m_sl = slice(mj * 128, (mj + 1) * 128)
                    kp = psum.tile([128, TS], F32, tag="kp")
                    pp = psum.tile([128, TS], F32, tag="pp")
                    for k in range(NK):
                        nc.tensor.matmul(
                            kp[:],
                            lhsT=wz_sb[k][j][:, m_sl],
                            rhs=xs[k][:],
                            start=(k == 0),
                            stop=(k == NK - 1),
                        )
                    for k in range(NK):
                        nc.tensor.matmul(
                            pp[:],
                            lhsT=wh_sb[k][j][:, m_sl],
                            rhs=xs[k][:],
                            start=(k == 0),
                            stop=(k == NK - 1),
                        )
                    z = work.tile([128, TS], F32, tag="z")
                    nc.scalar.activation(
                        out=z[:], in_=kp[:], func=AF.Sigmoid, bias=bz_sb[:, m:m + 1]
                    )
                    sp = work.tile([128, TS], F32, tag="sp")
                    nc.scalar.activation(
                        out=sp[:], in_=pp[:], func=AF.Sigmoid, bias=bh_sb[:, m:m + 1]
                    )
                    rp = work.tile([128, TS], F32, tag="rp")
                    nc.scalar.activation(
                        out=rp[:], in_=pp[:], func=AF.Relu, bias=bh_sb[:, m:m + 1]
                    )
                    # a = 1 - z
                    a = work.tile([128, TS], F32, tag="a")
                    nc.vector.tensor_scalar(
                        out=a[:], in0=z[:], scalar1=-1.0, scalar2=1.0,
                        op0=OP.mult, op1=OP.add,
                    )
                    # g = min(sigmoid(p+bh), 0.5) + relu(p+bh)
                    g = work.tile([128, TS], F32, tag="g")
                    nc.vector.scalar_tensor_tensor(
                        out=g[:], in0=sp[:], scalar=0.5, in1=rp[:],
                        op0=OP.min, op1=OP.add,
                    )
                    # b = z * g  (GpSimd: keeps DVE under the PE roofline)
                    b = work.tile([128, TS], F32, tag="b")
                    nc.gpsimd.tensor_tensor(out=b[:], in0=z[:], in1=g[:], op=OP.mult)
                    # h_t = a_t * h_{t-1} + b_t along the free axis
                    h = hpool.tile([128, TS], F32, tag=f"h{m}")
                    init = 0.5 if s == 0 else h_prev[m][:, TS - 1:TS]
                    nc.vector.tensor_tensor_scan(
                        out=h[:], data0=a[:], data1=b[:], initial=init,
                        op0=OP.mult, op1=OP.add,
                    )
                    h_prev[m] = h
                    nc.sync.dma_start(out=hT_d.ap()[m * 128:(m + 1) * 128, ts_sl],
                                      in_=h[:])

    nc.compile()
    return nc


def kernel(x, Wz, bz, Wh, bh):
    x = np.ascontiguousarray(x, dtype=np.float32)
    key = "nc"
    if key not in _cache:
        _cache[key] = build_nc()
    nc = _cache[key]

    wzT = np.ascontiguousarray(Wz.T.astype(np.float32))
    whT = np.ascontiguousarray(Wh.T.astype(np.float32))
    bz = np.ascontiguousarray(bz, dtype=np.float32)
    bh = np.ascontiguousarray(bh, dtype=np.float32)
    in_maps = [
        {
            "xT": np.ascontiguousarray(x[b].T),
            "wzT": wzT,
            "whT": whT,
            "bz": bz,
            "bh": bh,
        }
        for b in range(N_CORES)
    ]
    res = run_bass_kernel_spmd(nc, in_maps, list(range(N_CORES)))
    out = np.empty((B, S, H), np.float32)
    for b in range(N_CORES):
        out[b] = res.results[b]["hT"].T
    return out
